# revision 1
# baseline (speedup 1.0000x reference)
"""AttentionPool kernel for Trainium2, 8 NeuronCores (SPMD data-parallel).

Reference computation (per graph g with atoms A_g, uniform |A_g| = 32):
    h = X @ W.T                              [131072, 512]
    s = leakyrelu(sum(att * h, -1), 0.2)     [131072]
    w = segment_softmax(s)                   per graph
    out[g] = sum_{a in A_g} w[a] * h[a]      [4096, 512]

Algebraic refactor (pool-first; avoids the 69-GFLOP h matmul AND any
transpose of X):
    v  = W.T @ att  (host input prep, tiny)
    s  = lrelu(X @ v)        fused per-tile dot product on DVE
                             (scalar_tensor_tensor with accum_out)
    e  = exp(s)              ACT; no max-subtraction needed (|s| <~ 8)
    P[g] = sum_{a in A_g} e[a] X[a]   PE matmul per 128-atom tile with a
                             block-diagonal masked-exp stationary (built by
                             ACT: exp(maskbias + s), maskbias -1e9 off-block)
    d[g] = sum e[a]          one batched PE matmul per 8 tiles (E_b [128,32])
    out = (P / d) @ W.T      tiny per-core projection (PE transposes + matmul)

All matmuls in fp32 (fp32r = E8M11 measured ~1.4e-4 rel err -> rejected).
Per-core: 128 tiles; PE ~141us busy (the fp32 pool stream is the floor),
DVE ~105us (scores), DMA ~33MB. Measured ~170us on hardware, rel err 4.1e-6.

Sharding: 8 cores x 16384 atoms (= 512 graphs, graph-aligned). W/att
replicated. Output slices concatenated on host. Non-uniform segment sizes
fall back to an exact numpy path (never triggered by the fixed harness
inputs, which are uniform 32 atoms/graph).
"""

import numpy as np

N_ATOMS = 131072
FEAT = 512
N_GRAPHS = 4096
NEG_SLOPE = 0.2
N_CORES = 8

P = 128                      # partitions / atoms per tile
NA_CORE = N_ATOMS // N_CORES         # 16384 atoms per core
NT = NA_CORE // P                    # 128 tiles per core
NG_CORE = N_GRAPHS // N_CORES        # 512 graphs per core
GPT = P // 32                        # 4 graphs per tile (uniform 32 atoms/graph)
TPG = P // GPT                       # 32 tiles per 128-graph group
NGRP = NT // TPG                     # 4 groups of 128 graphs per core
FCH = FEAT // P                      # 4 feature chunks
DMA_GRP = 8                          # X tiles per input DMA (2 MiB)

_CACHED = {}


def _build_program():
    import concourse.bacc as bacc
    import concourse.mybir as mybir
    import concourse.tile as tile
    from concourse.masks import make_identity
    from contextlib import ExitStack

    F32 = mybir.dt.float32
    F32R = mybir.dt.float32r
    MULT = mybir.AluOpType.mult
    ADD = mybir.AluOpType.add
    MAX = mybir.AluOpType.max
    EXP = mybir.ActivationFunctionType.Exp

    nc = bacc.Bacc("TRN2", target_bir_lowering=False, debug=False,
                   num_devices=N_CORES)

    x = nc.dram_tensor("x", [NA_CORE, FEAT], F32, kind="ExternalInput").ap()
    wt = nc.dram_tensor("wt", [FEAT, FEAT], F32, kind="ExternalInput").ap()
    vrep = nc.dram_tensor("vrep", [P, FEAT], F32, kind="ExternalInput").ap()
    mb2 = nc.dram_tensor("mb2", [P, 2 * P - GPT], F32, kind="ExternalInput").ap()
    mask4 = nc.dram_tensor("mask4", [P, GPT], F32, kind="ExternalInput").ap()
    out = nc.dram_tensor("out", [NG_CORE, FEAT], F32, kind="ExternalOutput").ap()

    x_r = x.rearrange("(n o p) f -> n p o f", o=DMA_GRP, p=P)  # [NT/4, 128, 4, 512]

    with tile.TileContext(nc) as tc, ExitStack() as ctx:
        singles = ctx.enter_context(tc.tile_pool(name="singles", bufs=1))
        xpool = ctx.enter_context(tc.tile_pool(name="xpool", bufs=6))
        fxpool = ctx.enter_context(tc.tile_pool(name="fxpool", bufs=16))
        spool = ctx.enter_context(tc.tile_pool(name="spool", bufs=4))
        ttpool = ctx.enter_context(tc.tile_pool(name="ttpool", bufs=2))
        empool = ctx.enter_context(tc.tile_pool(name="empool", bufs=6))
        ebpool = ctx.enter_context(tc.tile_pool(name="ebpool", bufs=3))
        smallp = ctx.enter_context(tc.tile_pool(name="smallp", bufs=4))
        pooledp = ctx.enter_context(tc.tile_pool(name="pooledp", bufs=2))
        ptp = ctx.enter_context(tc.tile_pool(name="ptp", bufs=4))
        outp = ctx.enter_context(tc.tile_pool(name="outp", bufs=2))
        ps_pool = ctx.enter_context(tc.tile_pool(name="ps_pool", bufs=2, space="PSUM"))
        ps_den = ctx.enter_context(tc.tile_pool(name="ps_den", bufs=2, space="PSUM"))
        ps_misc = ctx.enter_context(tc.tile_pool(name="ps_misc", bufs=2, space="PSUM"))
        ps_out = ctx.enter_context(tc.tile_pool(name="ps_out", bufs=2, space="PSUM"))

        # ---- constants / weights (small DMAs first, X streaming starts async) ----
        v_rep = singles.tile([P, FEAT], F32)
        nc.sync.dma_start(out=v_rep, in_=vrep)
        # prefetch the first 16 X tiles as individual 256KB DMAs so the score
        # pipeline starts ASAP, before the 1MB wt load hogs the queue
        x_t = x.rearrange("(t p) f -> t p f", p=P)       # [NT, 128, 512]
        first_x = []
        for t in range(4):
            x1 = fxpool.tile([P, FEAT], F32, tag="x1", name=f"x1_{t}")
            nc.sync.dma_start(out=x1, in_=x_t[t])
            first_x.append(x1)
        mb2_sb = singles.tile([P, 2 * P - GPT], F32)
        nc.sync.dma_start(out=mb2_sb, in_=mb2)
        mask4_sb = singles.tile([P, GPT], F32)
        nc.sync.dma_start(out=mask4_sb, in_=mask4)
        for t in range(4, 16):
            x1 = fxpool.tile([P, FEAT], F32, tag="x1", name=f"x1_{t}")
            nc.sync.dma_start(out=x1, in_=x_t[t])
            first_x.append(x1)
        wt_sb = singles.tile([P, FCH, FEAT], F32)
        nc.sync.dma_start(out=wt_sb, in_=wt.rearrange("(c p) f -> p c f", p=P))
        ident = singles.tile([P, P], F32)
        make_identity(nc, ident)
        ones_col = singles.tile([P, 1], F32)
        nc.vector.memset(ones_col, 1.0)

        # warm up the PE (HAM clock gate) while the score pipeline fills:
        # dummy matmuls on v_rep keep TensorE busy from ~7us so the first
        # real pool matmul runs at 2.4GHz instead of 1.2
        warm_ps = ps_misc.tile([P, FEAT], F32, tag="misc", name="warm_ps")
        for wi in range(6):
            nc.tensor.matmul(warm_ps, lhsT=ident, rhs=v_rep,
                             start=(wi == 0), stop=(wi == 5))

        # ---- main loop over 4 groups x 32 tiles ----
        for g in range(NGRP):
            pool_ps = ps_pool.tile([P, FEAT], F32)
            den_ps = ps_den.tile([P, 1], F32)
            E_g = ebpool.tile([P, P], F32, tag="E_g")
            for bu in range(TPG // 8):           # 4 batches of 8 tiles
                s_b = spool.tile([P, 8], F32, tag="s_b")
                xts = []
                for k in range(8):
                    t = g * TPG + bu * 8 + k
                    if t < 16:
                        xts.append(first_x[t])
                    else:
                        n, o = divmod(t, DMA_GRP)
                        if o == 0:
                            x4 = xpool.tile([P, DMA_GRP, FEAT], F32, tag="x4")
                            nc.sync.dma_start(out=x4, in_=x_r[n])
                        xts.append(x4[:, o, :])
                    tt_out = ttpool.tile([P, FEAT], F32, tag="tt")
                    nc.vector.scalar_tensor_tensor(
                        out=tt_out, in0=xts[k], scalar=1.0, in1=v_rep,
                        op0=MULT, op1=MULT,
                        accum_out=s_b[:, k:k + 1])
                s_lr = spool.tile([P, 8], F32, tag="s_lr")
                nc.vector.scalar_tensor_tensor(
                    out=s_lr, in0=s_b, scalar=NEG_SLOPE, in1=s_b,
                    op0=MULT, op1=MAX)
                # e_b = exp(s_lr); E_b[p, 4k+j] = e_b[p, k] * mask4[p, j]
                e_b = spool.tile([P, 8], F32, tag="e_b")
                nc.scalar.activation(out=e_b, in_=s_lr, func=EXP, scale=1.0)
                E_gv = E_g.rearrange("p (u j) -> p u j", j=4)
                for j in range(4):
                    nc.vector.tensor_scalar_mul(
                        E_gv[:, bu * 8:bu * 8 + 8, j],
                        e_b, mask4_sb[:, j:j + 1])
                for k in range(8):
                    u = bu * 8 + k
                    em = empool.tile([P, P], F32, tag="em")
                    nc.scalar.activation(out=em,
                                         in_=mb2_sb[:, P - GPT - GPT * u:
                                                    2 * P - GPT - GPT * u],
                                         func=EXP,
                                         bias=s_lr[:, k:k + 1], scale=1.0)
                    nc.tensor.matmul(pool_ps, lhsT=em,
                                     rhs=xts[k],
                                     start=(u == 0), stop=(u == TPG - 1))
            # one denominator matmul per group: den[4u+j] = sum_p E_g[p, 4u+j]
            nc.tensor.matmul(den_ps, lhsT=E_g, rhs=ones_col,
                             start=True, stop=True)
            # normalize per f-chunk so each transpose can start immediately
            denr = smallp.tile([P, 1], F32, tag="denr")
            nc.vector.reciprocal(denr, den_ps)
            pooled = pooledp.tile([P, FEAT], F32, tag="pooled")

            # ---- projection: out[g] = pooled @ W.T ----
            out_ps = ps_out.tile([P, FEAT], F32)
            for c in range(FCH):
                nc.vector.tensor_scalar_mul(pooled[:, c * P:(c + 1) * P],
                                            pool_ps[:, c * P:(c + 1) * P], denr)
                tr_full = ps_misc.tile([P, FEAT], F32, tag="misc", name="tr_full")
                tr_ps = tr_full[:, :P]
                nc.tensor.transpose(tr_ps,
                                    pooled[:, c * P:(c + 1) * P],
                                    ident)
                pt = ptp.tile([P, P], F32, tag="pt")
                nc.scalar.copy(out=pt, in_=tr_ps)
                nc.tensor.matmul(out_ps, lhsT=pt,
                                 rhs=wt_sb[:, c, :],
                                 start=(c == 0), stop=(c == FCH - 1))
            out_sb = outp.tile([P, FEAT], F32, tag="out_sb")
            nc.scalar.copy(out=out_sb, in_=out_ps)
            nc.sync.dma_start(out=out[g * P:(g + 1) * P, :], in_=out_sb)
    nc.compile()
    return nc


def _host_inputs(atomwise_output, W, att_weight):
    """Per-core input maps (host-side prep is cheap reshapes only)."""
    X = np.ascontiguousarray(atomwise_output, dtype=np.float32)
    Wc = np.ascontiguousarray(W, dtype=np.float32)
    Wt = np.ascontiguousarray(Wc.T)
    att = np.asarray(att_weight, dtype=np.float32)
    v = Wt @ att                                               # v = W.T @ att
    vrep = np.ascontiguousarray(np.broadcast_to(v, (P, FEAT))).astype(np.float32)
    # master mask-bias: mb2[p, c] = 0 iff c == (P - GPT) + p//32; the per-tile
    # variant u is the window mb2[:, (P-GPT) - GPT*u : (2P-GPT) - GPT*u]
    pp = np.arange(P)[:, None]
    cc = np.arange(2 * P - GPT)[None, :]
    mb2 = np.where(cc == (P - GPT) + pp // 32, 0.0, -1e9).astype(np.float32)
    mb2 = np.ascontiguousarray(mb2)
    mask4 = (np.arange(P)[:, None] // 32 == np.arange(GPT)[None, :]).astype(np.float32)
    in_maps = []
    for c in range(N_CORES):
        xc = np.ascontiguousarray(X[c * NA_CORE:(c + 1) * NA_CORE])
        in_maps.append({"x": xc, "wt": Wt, "vrep": vrep, "mb2": mb2,
                        "mask4": mask4})
    return in_maps


def _kernel_numpy_fallback(atomwise_output, n_atoms_i, W, att_weight):
    """Exact reference semantics in numpy (used only for non-uniform segments)."""
    X = np.asarray(atomwise_output, dtype=np.float32)
    n_at = np.asarray(n_atoms_i).astype(np.int64)
    W = np.asarray(W, dtype=np.float32)
    att = np.asarray(att_weight, dtype=np.float32)
    h = X @ W.T
    s = (att * h).sum(-1)
    s = np.where(s >= 0, s, NEG_SLOPE * s)
    seg = np.repeat(np.arange(len(n_at)), n_at)[:len(s)]
    ngr = len(n_at)
    smax = np.full(ngr, -np.inf, dtype=np.float32)
    np.maximum.at(smax, seg, s)
    e = np.exp(s - smax[seg])
    den = np.zeros(ngr, dtype=np.float32)
    np.add.at(den, seg, e)
    wgt = e / den[seg]
    outp = np.zeros((ngr, h.shape[1]), dtype=np.float32)
    np.add.at(outp, seg, wgt[:, None] * h)
    return outp


def _run_on_device(atomwise_output, W, att_weight):
    from concourse.bass_utils import run_bass_kernel_spmd

    if "nc" not in _CACHED:
        _CACHED["nc"] = _build_program()
    nc = _CACHED["nc"]
    in_maps = _host_inputs(atomwise_output, W, att_weight)
    res = run_bass_kernel_spmd(nc, in_maps, list(range(N_CORES)))
    return np.concatenate([res.results[c]["out"] for c in range(N_CORES)], axis=0)


def _run_in_subprocess(atomwise_output, n_atoms_i, W, att_weight):
    """Last-resort retry in a fresh process: a transient
    NRT_EXEC_UNIT_UNRECOVERABLE wedges the current NRT client session, but a
    new process (fresh axon boot) recovers. Arrays go via a temp dir."""
    import os, subprocess, sys, tempfile
    kdir = os.path.dirname(os.path.abspath(__file__))
    with tempfile.TemporaryDirectory() as td:
        np.save(os.path.join(td, "x.npy"), np.asarray(atomwise_output))
        np.save(os.path.join(td, "n.npy"), np.asarray(n_atoms_i))
        np.save(os.path.join(td, "w.npy"), np.asarray(W))
        np.save(os.path.join(td, "a.npy"), np.asarray(att_weight))
        driver = (
            "import sys, os, numpy as np\n"
            f"sys.path.insert(0, {kdir!r})\n"
            "import kernel\n"
            f"td = {td!r}\n"
            "out = kernel.kernel(np.load(td+'/x.npy'), np.load(td+'/n.npy'),\n"
            "                    np.load(td+'/w.npy'), np.load(td+'/a.npy'))\n"
            "np.save(td+'/out.npy', out)\n"
        )
        env = dict(os.environ, KERNEL_NO_SUBPROC="1")
        subprocess.run([sys.executable, "-c", driver], env=env, check=True,
                       timeout=1800)
        return np.load(os.path.join(td, "out.npy"))


def kernel(atomwise_output, n_atoms_i, W, att_weight):
    import os
    n_at = np.asarray(n_atoms_i)
    uniform = (
        atomwise_output.shape == (N_ATOMS, FEAT)
        and n_at.shape == (N_GRAPHS,)
        and np.all(n_at == N_ATOMS // N_GRAPHS)
    )
    if not uniform:
        return _kernel_numpy_fallback(atomwise_output, n_atoms_i, W, att_weight)

    try:
        out = _run_on_device(atomwise_output, W, att_weight)
    except Exception:
        try:
            out = _run_on_device(atomwise_output, W, att_weight)
        except Exception:
            if os.environ.get("KERNEL_NO_SUBPROC"):
                raise
            out = _run_in_subprocess(atomwise_output, n_atoms_i, W, att_weight)
    return out.astype(np.float32)



# revision 8
# speedup vs baseline: 1.3874x; 1.3874x over previous
"""AttentionPool kernel for Trainium2, 8 NeuronCores (SPMD data-parallel).

Reference computation (per graph g with atoms A_g, uniform |A_g| = 32):
    h = X @ W.T                              [131072, 512]
    s = leakyrelu(sum(att * h, -1), 0.2)     [131072]
    w = segment_softmax(s)                   per graph
    out[g] = sum_{a in A_g} w[a] * h[a]      [4096, 512]

Algebraic refactor (pool-first; avoids the 69-GFLOP h matmul AND any
transpose of X):
    v  = W.T @ att  (host input prep, tiny)
    s  = lrelu(X @ v)        fused per-tile dot product (DVE/GpSimd stt)
    e  = exp(s)              no max-subtraction needed (|s| <~ 8)
    P[g] = sum_{a in A_g} e[a] X[a]   PE matmul per 128-atom tile with a
                             [128,32] masked-exp stationary em32 built by
                             ACT: exp(maskbias + s); 8-tile batches write
                             a 32-aligned PSUM partition window
    d[g] = per-tile matmul em32.T @ ones (ap_size=1, ~free on PE)
    out = (P / d) @ W.T      per-core projection (PE transposes + matmul)

All heavy data in bf16 (X converted host-side -> 17 MB DMA per core,
matmuls at 1 cyc/row); s/den/PSUM accumulate in fp32. Rel err ~1e-3
vs the 2e-2 gate.

Sharding: 8 cores x 16384 atoms (= 512 graphs, graph-aligned). W/att
replicated. Output slices concatenated on host. Non-uniform segment sizes
fall back to an exact numpy path (never triggered by the fixed harness
inputs, which are uniform 32 atoms/graph).
"""

import numpy as np

N_ATOMS = 131072
FEAT = 512
N_GRAPHS = 4096
NEG_SLOPE = 0.2
N_CORES = 8

P = 128                      # partitions / atoms per tile
NA_CORE = N_ATOMS // N_CORES         # 16384 atoms per core
NT = NA_CORE // P                    # 128 tiles per core
NG_CORE = N_GRAPHS // N_CORES        # 512 graphs per core
GPT = P // 32                        # 4 graphs per tile (uniform 32 atoms/graph)
TPG = P // GPT                       # 32 tiles per 128-graph group
NGRP = NT // TPG                     # 4 groups of 128 graphs per core
FCH = FEAT // P                      # 4 feature chunks
DMA_GRP = 8                          # X tiles per input DMA (1 MiB in bf16)
W32 = 8 * GPT                        # stationary width = graphs per 8-tile batch
GP_KS = ()                           # gpsimd lacks TensorScalarPtr: scores all-DVE

_CACHED = {}


def _build_program():
    import concourse.bacc as bacc
    import concourse.mybir as mybir
    import concourse.tile as tile
    from concourse.masks import make_identity
    from contextlib import ExitStack

    F32 = mybir.dt.float32
    BF16 = mybir.dt.bfloat16
    MULT = mybir.AluOpType.mult
    MAX = mybir.AluOpType.max
    EXP = mybir.ActivationFunctionType.Exp

    nc = bacc.Bacc("TRN2", target_bir_lowering=False, debug=False,
                   num_devices=N_CORES)

    x = nc.dram_tensor("x", [NA_CORE, FEAT], BF16, kind="ExternalInput").ap()
    wt = nc.dram_tensor("wt", [FEAT, FEAT], BF16, kind="ExternalInput").ap()
    vrep = nc.dram_tensor("vrep", [P, FEAT], BF16, kind="ExternalInput").ap()
    mb32 = nc.dram_tensor("mb32", [P, 2 * W32 - GPT], F32,
                          kind="ExternalInput").ap()
    out = nc.dram_tensor("out", [NG_CORE, FEAT], F32, kind="ExternalOutput").ap()

    x_r = x.rearrange("(n o p) f -> n p o f", o=DMA_GRP, p=P)

    with tile.TileContext(nc) as tc, ExitStack() as ctx:
        singles = ctx.enter_context(tc.tile_pool(name="singles", bufs=1))
        xpool = ctx.enter_context(tc.tile_pool(name="xpool", bufs=6))
        fxpool = ctx.enter_context(tc.tile_pool(name="fxpool", bufs=16))
        spool = ctx.enter_context(tc.tile_pool(name="spool", bufs=6))
        ttpool = ctx.enter_context(tc.tile_pool(name="ttpool", bufs=4))
        empool = ctx.enter_context(tc.tile_pool(name="empool", bufs=8))
        smallp = ctx.enter_context(tc.tile_pool(name="smallp", bufs=4))
        pooledp = ctx.enter_context(tc.tile_pool(name="pooledp", bufs=2))
        ptp = ctx.enter_context(tc.tile_pool(name="ptp", bufs=4))
        outp = ctx.enter_context(tc.tile_pool(name="outp", bufs=2))
        ps_pool = ctx.enter_context(tc.tile_pool(name="ps_pool", bufs=2, space="PSUM"))
        ps_den = ctx.enter_context(tc.tile_pool(name="ps_den", bufs=2, space="PSUM"))
        ps_misc = ctx.enter_context(tc.tile_pool(name="ps_misc", bufs=2, space="PSUM"))
        ps_out = ctx.enter_context(tc.tile_pool(name="ps_out", bufs=2, space="PSUM"))

        # ---- constants / weights (small DMAs first, X streaming starts async) ----
        v_rep = singles.tile([P, FEAT], BF16)
        nc.sync.dma_start(out=v_rep, in_=vrep)
        # prefetch the first 16 X tiles as individual 128KB DMAs so the score
        # pipeline starts ASAP, before the 512KB wt load hogs the queue
        x_t = x.rearrange("(t p) f -> t p f", p=P)       # [NT, 128, 512]
        first_x = []
        for t in range(4):
            x1 = fxpool.tile([P, FEAT], BF16, tag="x1", name=f"x1_{t}")
            nc.sync.dma_start(out=x1, in_=x_t[t])
            first_x.append(x1)
        mb32_sb = singles.tile([P, 2 * W32 - GPT], F32)
        nc.sync.dma_start(out=mb32_sb, in_=mb32)
        for t in range(4, 16):
            x1 = fxpool.tile([P, FEAT], BF16, tag="x1", name=f"x1_{t}")
            nc.sync.dma_start(out=x1, in_=x_t[t])
            first_x.append(x1)
        wt_sb = singles.tile([P, FCH, FEAT], BF16)
        nc.sync.dma_start(out=wt_sb, in_=wt.rearrange("(c p) f -> p c f", p=P))
        ident = singles.tile([P, P], BF16)
        make_identity(nc, ident)
        ones_col = singles.tile([P, 1], BF16)
        nc.vector.memset(ones_col, 1.0)

        # warm up the PE (HAM clock gate) while the score pipeline fills
        warm_ps = ps_out.tile([P, FEAT], F32, tag="ops", name="warm_ps")
        for wi in range(6):
            nc.tensor.matmul(warm_ps, lhsT=ident, rhs=v_rep,
                             start=(wi == 0), stop=(wi == 5))

        # ---- main loop over 4 groups x 4 batches x 8 tiles ----
        for g in range(NGRP):
            pool_ps = ps_pool.tile([P, FEAT], F32)
            den_ps = ps_den.tile([P, 1], F32)
            for bu in range(TPG // 8):           # 4 batches of 8 tiles
                s_b = spool.tile([P, 8], F32, tag="s_b")
                xts = []
                for k in range(8):
                    t = g * TPG + bu * 8 + k
                    if t < 16:
                        xts.append(first_x[t])
                    else:
                        n, o = divmod(t, DMA_GRP)
                        if o == 0:
                            x4 = xpool.tile([P, DMA_GRP, FEAT], BF16, tag="x4")
                            nc.sync.dma_start(out=x4, in_=x_r[n])
                        xts.append(x4[:, o, :])
                    tt_out = ttpool.tile([P, FEAT], BF16, tag="tt")
                    eng = nc.gpsimd if k in GP_KS else nc.vector
                    eng.scalar_tensor_tensor(
                        out=tt_out, in0=xts[k], scalar=1.0, in1=v_rep,
                        op0=MULT, op1=MULT,
                        accum_out=s_b[:, k:k + 1])
                s_lr = spool.tile([P, 8], F32, tag="s_lr")
                nc.vector.scalar_tensor_tensor(
                    out=s_lr, in0=s_b, scalar=NEG_SLOPE, in1=s_b,
                    op0=MULT, op1=MAX)
                win = pool_ps[bu * W32:(bu + 1) * W32, :]
                dwin = den_ps[bu * W32:(bu + 1) * W32, :]
                for k in range(8):
                    # em32[p, c] = exp(s_lr[p,k]) iff c == 4k + p//32 else 0
                    em = empool.tile([P, W32], BF16, tag="em")
                    nc.scalar.activation(out=em,
                                         in_=mb32_sb[:, W32 - GPT - GPT * k:
                                                     2 * W32 - GPT - GPT * k],
                                         func=EXP,
                                         bias=s_lr[:, k:k + 1], scale=1.0)
                    nc.tensor.matmul(win, lhsT=em, rhs=xts[k],
                                     start=(k == 0), stop=(k == 7),
                                     tile_position=(0, bu * W32))
                    nc.tensor.matmul(dwin, lhsT=em, rhs=ones_col,
                                     start=(k == 0), stop=(k == 7),
                                     tile_position=(0, bu * W32))
            # ---- normalize + projection: out[g] = (pool/den) @ W.T ----
            denr = smallp.tile([P, 1], F32, tag="denr")
            nc.vector.reciprocal(denr, den_ps)
            pooled = pooledp.tile([P, FEAT], BF16, tag="pooled")
            out_ps = ps_out.tile([P, FEAT], F32, tag="ops")
            for c in range(FCH):
                nc.scalar.mul(pooled[:, c * P:(c + 1) * P],
                              pool_ps[:, c * P:(c + 1) * P], denr)
                tr_ps = ps_misc.tile([P, P], BF16, tag="tr", name="tr_ps")
                nc.tensor.transpose(tr_ps,
                                    pooled[:, c * P:(c + 1) * P],
                                    ident)
                pt = ptp.tile([P, P], BF16, tag="pt")
                nc.scalar.copy(out=pt, in_=tr_ps)
                nc.tensor.matmul(out_ps, lhsT=pt,
                                 rhs=wt_sb[:, c, :],
                                 start=(c == 0), stop=(c == FCH - 1))
            out_sb = outp.tile([P, FEAT], F32, tag="out_sb")
            nc.scalar.copy(out=out_sb, in_=out_ps)
            nc.sync.dma_start(out=out[g * P:(g + 1) * P, :], in_=out_sb)
    nc.compile()
    return nc


def _host_inputs(atomwise_output, W, att_weight):
    """Per-core input maps (host prep: bf16 conversion + tiny mask tables)."""
    import ml_dtypes
    BF = ml_dtypes.bfloat16
    X = np.asarray(atomwise_output, dtype=np.float32)
    Xb = X.astype(BF)
    Wc = np.ascontiguousarray(np.asarray(W, dtype=np.float32))
    Wt = np.ascontiguousarray(Wc.T)
    att = np.asarray(att_weight, dtype=np.float32)
    v = Wt @ att                                               # v = W.T @ att
    Wtb = Wt.astype(BF)
    vrep = np.ascontiguousarray(np.broadcast_to(v, (P, FEAT))).astype(BF)
    # master mask-bias: mb32[p, c] = 0 iff c == (W32 - GPT) + p//32; the
    # per-tile variant k is the window mb32[:, (W32-GPT)-GPT*k : (2*W32-GPT)-GPT*k]
    pp = np.arange(P)[:, None]
    cc = np.arange(2 * W32 - GPT)[None, :]
    mb = np.where(cc == (W32 - GPT) + pp // 32, 0.0, -1e9).astype(np.float32)
    mb = np.ascontiguousarray(mb)
    in_maps = []
    for c in range(N_CORES):
        xc = Xb[c * NA_CORE:(c + 1) * NA_CORE]
        in_maps.append({"x": xc, "wt": Wtb, "vrep": vrep, "mb32": mb})
    return in_maps


def _kernel_numpy_fallback(atomwise_output, n_atoms_i, W, att_weight):
    """Exact reference semantics in numpy (used only for non-uniform segments)."""
    X = np.asarray(atomwise_output, dtype=np.float32)
    n_at = np.asarray(n_atoms_i).astype(np.int64)
    W = np.asarray(W, dtype=np.float32)
    att = np.asarray(att_weight, dtype=np.float32)
    h = X @ W.T
    s = (att * h).sum(-1)
    s = np.where(s >= 0, s, NEG_SLOPE * s)
    seg = np.repeat(np.arange(len(n_at)), n_at)[:len(s)]
    ngr = len(n_at)
    smax = np.full(ngr, -np.inf, dtype=np.float32)
    np.maximum.at(smax, seg, s)
    e = np.exp(s - smax[seg])
    den = np.zeros(ngr, dtype=np.float32)
    np.add.at(den, seg, e)
    wgt = e / den[seg]
    outp = np.zeros((ngr, h.shape[1]), dtype=np.float32)
    np.add.at(outp, seg, wgt[:, None] * h)
    return outp


def _run_on_device(atomwise_output, W, att_weight):
    from concourse.bass_utils import run_bass_kernel_spmd

    if "nc" not in _CACHED:
        _CACHED["nc"] = _build_program()
    nc = _CACHED["nc"]
    in_maps = _host_inputs(atomwise_output, W, att_weight)
    res = run_bass_kernel_spmd(nc, in_maps, list(range(N_CORES)))
    return np.concatenate([res.results[c]["out"] for c in range(N_CORES)], axis=0)


def _run_in_subprocess(atomwise_output, n_atoms_i, W, att_weight):
    """Last-resort retry in a fresh process: a transient
    NRT_EXEC_UNIT_UNRECOVERABLE wedges the current NRT client session, but a
    new process (fresh axon boot) recovers. Arrays go via a temp dir."""
    import os, subprocess, sys, tempfile
    kdir = os.path.dirname(os.path.abspath(__file__))
    with tempfile.TemporaryDirectory() as td:
        np.save(os.path.join(td, "x.npy"), np.asarray(atomwise_output))
        np.save(os.path.join(td, "n.npy"), np.asarray(n_atoms_i))
        np.save(os.path.join(td, "w.npy"), np.asarray(W))
        np.save(os.path.join(td, "a.npy"), np.asarray(att_weight))
        driver = (
            "import sys, os, numpy as np\n"
            f"sys.path.insert(0, {kdir!r})\n"
            "import kernel\n"
            f"td = {td!r}\n"
            "out = kernel.kernel(np.load(td+'/x.npy'), np.load(td+'/n.npy'),\n"
            "                    np.load(td+'/w.npy'), np.load(td+'/a.npy'))\n"
            "np.save(td+'/out.npy', out)\n"
        )
        env = dict(os.environ, KERNEL_NO_SUBPROC="1")
        subprocess.run([sys.executable, "-c", driver], env=env, check=True,
                       timeout=1800)
        return np.load(os.path.join(td, "out.npy"))


def kernel(atomwise_output, n_atoms_i, W, att_weight):
    import os
    n_at = np.asarray(n_atoms_i)
    uniform = (
        atomwise_output.shape == (N_ATOMS, FEAT)
        and n_at.shape == (N_GRAPHS,)
        and np.all(n_at == N_ATOMS // N_GRAPHS)
    )
    if not uniform:
        return _kernel_numpy_fallback(atomwise_output, n_atoms_i, W, att_weight)

    try:
        out = _run_on_device(atomwise_output, W, att_weight)
    except Exception:
        try:
            out = _run_on_device(atomwise_output, W, att_weight)
        except Exception:
            if os.environ.get("KERNEL_NO_SUBPROC"):
                raise
            out = _run_in_subprocess(atomwise_output, n_atoms_i, W, att_weight)
    return out.astype(np.float32)
